# revision 15
# baseline (speedup 1.0000x reference)
"""Trainium2 Bass kernel for nn_ClassifierGNN (gnn_message_passing).

Strategy (8 NeuronCores, rows of the NxN pairwise tensor sharded 96/core):
  Pass A: per-core covariance C = sum_ij x x^T and mean of x (bf16 on PE)
          -> AllReduce -> BN0 stats computed analytically on device.
  Pass B: stream x_ij tiles -> conv0 (f32r) -> fused BN+LeakyReLU (ACT)
          -> conv1 (f32r) -> h1 kept resident in SBUF as bf16 + h1 stats
          -> AllReduce of BN1 stats.
  Pass C: BN1+LeakyReLU on h1, conv_out as column-writes building simT
          on PSUM, sigmoid -> per-core simT block.
  AllGather simT blocks; tail (edge normalize + NodeNet + its BNs)
  computed redundantly per core in transposed layout, fully local.
"""

import numpy as np

N, D, EF, C2, NC_ = 768, 128, 128, 256, 65
NCORES, R = 8, 96
F = 384  # j-chunk for pass B
EPS = 1e-5
NEG = 0.01

_cached = {}


def _build():
    import os
    from concourse import bacc, tile, mybir
    from concourse.dt import dt
    NOCOLL = bool(int(os.environ.get("KNOCOLL", "0")))
    STAGE = int(os.environ.get("KSTAGE", "4"))
    KPA = os.environ.get("KPA", "")

    AF = mybir.ActivationFunctionType
    OP = mybir.AluOpType
    f32, bf16, f32r = dt.float32, dt.bfloat16, dt.float32r
    AX = mybir.AxisListType.X

    nc = bacc.Bacc("TRN2", target_bir_lowering=False, debug=False,
                   num_devices=NCORES)

    def din(name, shape):
        return nc.dram_tensor(name, list(shape), mybir.dt.float32,
                              kind="ExternalInput")

    featT_in = din("featT", [128, N])
    featblk_in = din("featblk", [128, 6, 128])
    bcast_in = din("bcastrows", [R * 128, 128])
    myrowsT_in = din("myrowsT", [128, R])
    w0T_in = din("w0T", [128, 2, 128])
    w0_in = din("w0", [128, 2, 128])
    w1T_in = din("w1T", [128, 2, 128])
    bn0_in = din("bn0", [128, 2, 2])
    bn1_in = din("bn1", [128, 2])
    woutT_in = din("woutT", [128, 1])
    bout_in = din("bout", [1, 1])
    maskT_in = din("maskT", [128, 6, N])
    epT_in = din("epT", [128, 6, N])
    odT_in = din("odT", [128, 6, N])
    nw0T_in = din("nw0T", [128, 2, 130])
    nbn0_in = din("nbn0", [130, 2])
    nw1T_in = din("nw1T", [130, 65])
    nbn1_in = din("nbn1", [65, 2])
    ident_in = din("ident", [128, 128])

    logits_out = nc.dram_tensor("logits", [N, NC_], mybir.dt.float32,
                                kind="ExternalOutput")
    sim_out = nc.dram_tensor("sim", [N, N], mybir.dt.float32,
                             kind="ExternalOutput")

    # collective bounce buffers (internal DRAM)
    ar0_src = nc.dram_tensor("ar0_src", [128, 130], mybir.dt.float32)
    ar0_dst = nc.dram_tensor("ar0_dst", [128, 130], mybir.dt.float32)
    ar1_src = nc.dram_tensor("ar1_src", [128, 2], mybir.dt.float32)
    ar1_dst = nc.dram_tensor("ar1_dst", [128, 2], mybir.dt.float32)
    ag_src = nc.dram_tensor("ag_src", [N, R], mybir.dt.float32)
    ag_dst = nc.dram_tensor("ag_dst", [NCORES * N, R], mybir.dt.float32)

    NSQ = float(N) * float(N)

    from contextlib import ExitStack
    with tile.TileContext(nc) as tc:
      with tc.tile_pool(name="res", bufs=1) as res, \
           tc.tile_pool(name="stat", bufs=1) as stat:
        with tc.tile_pool(name="h1res", bufs=1) as h1pool, \
             tc.tile_pool(name="work", bufs=2) as work:
            pstack = ExitStack()
            psA = pstack.enter_context(
                tc.tile_pool(name="psA", bufs=1, space="PSUM"))
            ps = psA

            # ---------- resident loads ----------
            featT = res.tile([128, N], f32)
            featblk = res.tile([128, 6, 128], f32)
            myrowsT = res.tile([128, R], f32)
            w0T = res.tile([128, 2, 128], f32)
            w0r_ = res.tile([128, 2, 128], f32r)
            w0rows = res.tile([128, 2, 128], f32)
            w1T = res.tile([128, 2, 128], f32)
            w1r_ = res.tile([128, 2, 128], f32r)
            bn0 = res.tile([128, 2, 2], f32)
            bn1 = res.tile([128, 2], f32)
            woutT = res.tile([128, 1], f32)
            woutb = res.tile([128, 1], bf16)
            ident = res.tile([128, 128], f32)
            for t, src in [(featT, featT_in), (featblk, featblk_in),
                           (myrowsT, myrowsT_in),
                           (w0T, w0T_in), (w0rows, w0_in), (w1T, w1T_in),
                           (bn0, bn0_in), (bn1, bn1_in), (woutT, woutT_in),
                           (ident, ident_in)]:
                nc.sync.dma_start(out=t[:], in_=src[:])
            negrT = res.tile([128, R], f32)
            nc.vector.tensor_scalar(out=negrT[:], in0=myrowsT[:], scalar1=-1.0,
                                    scalar2=None, op0=OP.mult)
            for b in range(2):
                nc.vector.tensor_copy(out=w0r_[:, b, :], in_=w0T[:, b, :])
                nc.vector.tensor_copy(out=w1r_[:, b, :], in_=w1T[:, b, :])
            nc.vector.tensor_copy(out=woutb[:], in_=woutT[:])
            ones1 = res.tile([1, 128], f32)
            nc.vector.memset(ones1[:], 1.0)
            onesb = res.tile([128, 1], bf16)
            nc.vector.memset(onesb[:], 1.0)
            epscol = res.tile([128, 1], f32)
            nc.vector.memset(epscol[:], EPS)

            # ---------- Pass A: C = sum x x^T, mx = sum x  (bf16) ----------
            Cp = psA.tile([128, 128], f32)
            mxp = psA.tile([128, 1], f32)
            nmm = 0
            NACC = R * 6 if "shortacc" not in KPA else 12
            for i in range(R if STAGE >= 1 else 0):
                bcs = work.tile([128, 128], f32, tag="bcs")
                if "nodma" in KPA:
                    nc.vector.memset(bcs[:], 0.5)
                else:
                    nc.sync.dma_start(out=bcs[:], in_=bcast_in[128 * i:128 * (i + 1), :])
                for b in range(6):
                    xb = work.tile([128, 128], bf16, tag="xb")
                    if "nogen" in KPA:
                        nc.vector.memset(xb[:], 0.25)
                    else:
                        xs = work.tile([128, 128], f32, tag="xs")
                        nc.vector.tensor_tensor(out=xs[:], in0=featblk[:, b, :],
                                                in1=bcs[:], op=OP.subtract)
                        if (i * 6 + b) % 4 < 3:
                            nc.scalar.activation(out=xb[:], in_=xs[:], func=AF.Abs,
                                                 bias=0.0, scale=1.0)
                        else:
                            nc.vector.scalar_tensor_tensor(
                                out=xb[:], in0=xs[:], scalar=-1.0, in1=xs[:],
                                op0=OP.mult, op1=OP.max)
                    acc_i = nmm % NACC
                    first = acc_i == 0
                    last = (acc_i == NACC - 1) or (i == R - 1 and b == 5)
                    if "nomm" not in KPA:
                        nc.tensor.matmul(out=Cp[:], lhsT=xb[:], rhs=xb[:],
                                         start=first, stop=last)
                    if "nomx" not in KPA and "nomm" not in KPA:
                        nc.tensor.matmul(out=mxp[:], lhsT=xb[:], rhs=onesb[:],
                                         start=first, stop=last)
                    nmm += 1
            if "nomm" in KPA:
                nc.tensor.matmul(out=Cp[:], lhsT=onesb[:].bitcast(bf16), rhs=onesb[:],
                                 start=True, stop=True) if False else None
                nc.vector.memset(Cp[:], 0.5)
                nc.vector.memset(mxp[:], 0.5)
            elif "nomx" in KPA:
                nc.vector.memset(mxp[:], 0.5)
            ar0sb = stat.tile([128, 130], f32)
            nc.scalar.copy(out=ar0sb[:, 0:128], in_=Cp[:])
            nc.scalar.copy(out=ar0sb[:, 128:129], in_=mxp[:])
            nc.vector.memset(ar0sb[:, 129:130], 0.0)
            nc.sync.dma_start(out=ar0_src[:], in_=ar0sb[:])
            if NOCOLL:
                nc.sync.dma_start(out=ar0_dst[:], in_=ar0_src[:])
            else:
                nc.gpsimd.collective_compute(
                    "AllReduce", OP.add, replica_groups=[list(range(NCORES))],
                    ins=[ar0_src.ap().opt()], outs=[ar0_dst.ap().opt()])
            CR = stat.tile([128, 130], f32)
            nc.sync.dma_start(out=CR[:], in_=ar0_dst[:])
            pstack.close()
            pstack = ExitStack()
            ps = pstack.enter_context(
                tc.tile_pool(name="ps0", bufs=1, space="PSUM"))

            # ---------- BN0 stats ----------
            s0 = stat.tile([128, 2], f32)
            t0 = stat.tile([128, 2], f32)
            for b in range(2):
                m0p = ps.tile([128, 1], f32, tag="m0p")
                nc.tensor.matmul(out=m0p[:], lhsT=w0T[:, b, :],
                                 rhs=CR[:, 128:129], start=True, stop=True)
                M1p = ps.tile([128, 128], f32, tag="M1p")
                nc.tensor.matmul(out=M1p[:], lhsT=w0T[:, b, :],
                                 rhs=CR[:, 0:128], start=True, stop=True)
                e2 = stat.tile([128, 1], f32, tag=f"e2_{b}")
                scr = work.tile([128, 128], f32, tag="scr")
                nc.vector.tensor_tensor(out=scr[:], in0=M1p[:],
                                        in1=w0rows[:, b, :], op=OP.mult)
                nc.vector.tensor_reduce(out=e2[:], in_=scr[:], axis=AX,
                                        op=OP.add)
                nc.vector.tensor_scalar(out=e2[:], in0=e2[:],
                                        scalar1=1.0 / NSQ, scalar2=None,
                                        op0=OP.mult)
                m0 = stat.tile([128, 1], f32, tag=f"m0_{b}")
                nc.vector.tensor_scalar(out=m0[:], in0=m0p[:],
                                        scalar1=1.0 / NSQ, scalar2=None,
                                        op0=OP.mult)
                msq = stat.tile([128, 1], f32, tag=f"msq_{b}")
                nc.vector.tensor_tensor(out=msq[:], in0=m0[:], in1=m0[:],
                                        op=OP.mult)
                var = stat.tile([128, 1], f32, tag=f"var_{b}")
                nc.vector.tensor_tensor(out=var[:], in0=e2[:], in1=msq[:],
                                        op=OP.subtract)
                sd = stat.tile([128, 1], f32, tag=f"sd_{b}")
                nc.scalar.activation(out=sd[:], in_=var[:], func=AF.Sqrt,
                                     bias=epscol[:], scale=1.0)
                inv = stat.tile([128, 1], f32, tag=f"inv_{b}")
                nc.vector.reciprocal(out=inv[:], in_=sd[:])
                # s = gamma * inv ; t = beta - m * s
                nc.vector.tensor_tensor(out=s0[:, b:b + 1], in0=bn0[:, b, 0:1],
                                        in1=inv[:], op=OP.mult)
                ms = stat.tile([128, 1], f32, tag=f"ms_{b}")
                nc.vector.tensor_tensor(out=ms[:], in0=m0[:],
                                        in1=s0[:, b:b + 1], op=OP.mult)
                nc.vector.tensor_tensor(out=t0[:, b:b + 1], in0=bn0[:, b, 1:2],
                                        in1=ms[:], op=OP.subtract)

            # ---------- Pass B ----------
            pstack.close()
            pstack = ExitStack()
            ps = pstack.enter_context(
                tc.tile_pool(name="psB", bufs=2, space="PSUM"))
            h1res = h1pool.tile([128, R, N], bf16)
            s1acc = stat.tile([128, 2 * R], f32)
            s2acc = stat.tile([128, 2 * R], f32)
            if STAGE < 2:
                nc.vector.memset(s1acc[:], 1.0)
                nc.vector.memset(s2acc[:], 2.0)
            for i in range(R if STAGE >= 2 else 0):
                for jc in range(2):
                    it = i * 2 + jc
                    xr = work.tile([128, F], f32r, tag="xr")
                    nc.scalar.activation(out=xr[:], in_=featT[:, jc * F:(jc + 1) * F],
                                         func=AF.Abs, bias=negrT[:, i:i + 1],
                                         scale=1.0)
                    h0p = [ps.tile([128, F], f32, tag="h0pa", name="h0pa"),
                           ps.tile([128, F], f32, tag="h0pb", name="h0pb")]
                    for b in range(2):
                        nc.tensor.matmul(out=h0p[b][:], lhsT=w0r_[:, b, :],
                                         rhs=xr[:], start=True, stop=True)
                    a0 = work.tile([128, 2, F], f32r, tag="a0")
                    for b in range(2):
                        if (it + b) % 2 == 0:
                            nc.scalar.activation(out=a0[:, b, :], in_=h0p[b][:],
                                                 func=AF.Lrelu, alpha=NEG,
                                                 bias=t0[:, b:b + 1],
                                                 scale=s0[:, b:b + 1])
                        else:
                            tmp = work.tile([128, F], f32, tag="a0tmp")
                            nc.vector.tensor_scalar(
                                out=tmp[:], in0=h0p[b][:],
                                scalar1=s0[:, b:b + 1], scalar2=t0[:, b:b + 1],
                                op0=OP.mult, op1=OP.add)
                            nc.vector.scalar_tensor_tensor(
                                out=a0[:, b, :], in0=tmp[:], scalar=NEG,
                                op0=OP.mult, op1=OP.max, in1=tmp[:])
                    h1p = ps.tile([128, F], f32, tag="h1p")
                    nc.tensor.matmul(out=h1p[:], lhsT=w1r_[:, 0, :],
                                     rhs=a0[:, 0, :], start=True, stop=False)
                    nc.tensor.matmul(out=h1p[:], lhsT=w1r_[:, 1, :],
                                     rhs=a0[:, 1, :], start=False, stop=True)
                    nc.scalar.activation(out=h1res[:, i, jc * F:(jc + 1) * F],
                                         in_=h1p[:], func=AF.Identity,
                                         bias=0.0, scale=1.0,
                                         accum_out=s1acc[:, it:it + 1])
                    sqs = work.tile([128, F], bf16, tag="sqs")
                    nc.scalar.activation(out=sqs[:],
                                         in_=h1res[:, i, jc * F:(jc + 1) * F],
                                         func=AF.Square, bias=0.0, scale=1.0,
                                         accum_out=s2acc[:, it:it + 1])

            ar1sb = stat.tile([128, 2], f32)
            nc.vector.tensor_reduce(out=ar1sb[:, 0:1], in_=s1acc[:], axis=AX,
                                    op=OP.add)
            nc.vector.tensor_reduce(out=ar1sb[:, 1:2], in_=s2acc[:], axis=AX,
                                    op=OP.add)
            nc.sync.dma_start(out=ar1_src[:], in_=ar1sb[:])
            if NOCOLL:
                nc.sync.dma_start(out=ar1_dst[:], in_=ar1_src[:])
            else:
                nc.gpsimd.collective_compute(
                    "AllReduce", OP.add, replica_groups=[list(range(NCORES))],
                    ins=[ar1_src.ap().opt()], outs=[ar1_dst.ap().opt()])
            SR = stat.tile([128, 2], f32)
            nc.sync.dma_start(out=SR[:], in_=ar1_dst[:])

            # BN1 stats
            s1 = stat.tile([128, 1], f32)
            t1 = stat.tile([128, 1], f32)
            m1 = stat.tile([128, 1], f32)
            nc.vector.tensor_scalar(out=m1[:], in0=SR[:, 0:1], scalar1=1.0 / NSQ,
                                    scalar2=None, op0=OP.mult)
            msq1 = stat.tile([128, 1], f32)
            nc.vector.tensor_tensor(out=msq1[:], in0=m1[:], in1=m1[:], op=OP.mult)
            var1 = stat.tile([128, 1], f32)
            nc.vector.tensor_scalar(out=var1[:], in0=SR[:, 1:2], scalar1=1.0 / NSQ,
                                    scalar2=None, op0=OP.mult)
            nc.vector.tensor_tensor(out=var1[:], in0=var1[:], in1=msq1[:],
                                    op=OP.subtract)
            sd1 = stat.tile([128, 1], f32)
            nc.scalar.activation(out=sd1[:], in_=var1[:], func=AF.Sqrt,
                                 bias=epscol[:], scale=1.0)
            inv1 = stat.tile([128, 1], f32)
            nc.vector.reciprocal(out=inv1[:], in_=sd1[:])
            nc.vector.tensor_tensor(out=s1[:], in0=bn1[:, 0:1], in1=inv1[:],
                                    op=OP.mult)
            ms1 = stat.tile([128, 1], f32)
            nc.vector.tensor_tensor(out=ms1[:], in0=m1[:], in1=s1[:], op=OP.mult)
            nc.vector.tensor_tensor(out=t1[:], in0=bn1[:, 1:2], in1=ms1[:],
                                    op=OP.subtract)

            pstack.close()
            pstack = ExitStack()
            ps = pstack.enter_context(
                tc.tile_pool(name="psC", bufs=2, space="PSUM"))

            # bout broadcast to [128,1]
            bout_sb1 = stat.tile([1, 1], f32)
            nc.sync.dma_start(out=bout_sb1[:], in_=bout_in[:])
            boutp = ps.tile([128, 1], f32, tag="boutp")
            nc.tensor.matmul(out=boutp[:], lhsT=ones1[:], rhs=bout_sb1[:],
                             start=True, stop=True)
            boutc = stat.tile([128, 1], f32)
            nc.scalar.copy(out=boutc[:], in_=boutp[:])

            # ---------- Pass C: simT = sigmoid(wout . lrelu(bn1(h1))) ----------
            simT = stat.tile([128, 6, R], f32)
            if STAGE < 3:
                nc.vector.memset(simT[:], 0.5)
            for bj in range(6 if STAGE >= 3 else 0):
                stp = ps.tile([128, R], f32, tag="stp")
                for i in range(R):
                    a1 = work.tile([128, 128], bf16, tag="a1")
                    nc.scalar.activation(out=a1[:],
                                         in_=h1res[:, i, bj * 128:(bj + 1) * 128],
                                         func=AF.Lrelu, alpha=NEG, bias=t1[:],
                                         scale=s1[:])
                    nc.tensor.matmul(out=stp[:, i:i + 1], lhsT=a1[:],
                                     rhs=woutb[:], start=True, stop=True)
                nc.scalar.activation(out=simT[:, bj, :], in_=stp[:],
                                     func=AF.Sigmoid, bias=boutc[:], scale=1.0)

            # ---------- AllGather simT ----------
            for b in range(6 if STAGE >= 3 else 0):
                nc.sync.dma_start(out=ag_src[128 * b:128 * (b + 1), :],
                                  in_=simT[:, b, :])
            if NOCOLL:
                nc.sync.dma_start(out=ag_dst[0:N, :], in_=ag_src[:])
            else:
                nc.gpsimd.collective_compute(
                    "AllGather", OP.bypass, replica_groups=[list(range(NCORES))],
                    ins=[ag_src.ap().opt()], outs=[ag_dst.ap().opt()])
            pstack.close()

        # ---------- tail (h1res freed) ----------
        with tc.tile_pool(name="tail", bufs=1) as tl, \
             tc.tile_pool(name="twork", bufs=3) as tw, \
             tc.tile_pool(name="tps", bufs=1, space="PSUM") as tps:
            TS = 6 if STAGE >= 4 else 0
            TS2 = 2 if STAGE >= 4 else 0
            simTf = tl.tile([128, 6, N], f32)
            for b in range(TS):
                for k in range(NCORES):
                    nc.sync.dma_start(
                        out=simTf[:, b, R * k:R * (k + 1)],
                        in_=ag_dst[N * k + 128 * b:N * k + 128 * (b + 1), :])
            maskT = tl.tile([128, 6, N], f32)
            epT = tl.tile([128, 6, N], f32)
            odT = tl.tile([128, 6, N], f32)
            nc.sync.dma_start(out=maskT[:], in_=maskT_in[:])
            nc.sync.dma_start(out=epT[:], in_=epT_in[:])
            nc.sync.dma_start(out=odT[:], in_=odT_in[:])

            # edge_pre = mask*sim + ep ; colsum over free(i) ; w = od*edge_pre/colsum[j]
            wT = tl.tile([128, 6, N], f32)
            rinv = tl.tile([128, 6], f32)
            colp = tl.tile([128, 6], f32)
            if STAGE < 4:
                for _t in (simTf, wT, rinv, colp):
                    nc.vector.memset(_t[:], 0.25)
            for b in range(TS):
                ep_ = tw.tile([128, N], f32, tag="ep_")
                nc.vector.tensor_tensor(out=ep_[:], in0=maskT[:, b, :],
                                        in1=simTf[:, b, :], op=OP.mult)
                nc.vector.tensor_tensor(out=ep_[:], in0=ep_[:], in1=epT[:, b, :],
                                        op=OP.add)
                nc.vector.tensor_reduce(out=colp[:, b:b + 1], in_=ep_[:],
                                        axis=AX, op=OP.add)
                cinv = tw.tile([128, 1], f32, tag="cinv")
                nc.vector.reciprocal(out=cinv[:], in_=colp[:, b:b + 1])
                od_ = tw.tile([128, N], f32, tag="od_")
                nc.vector.tensor_tensor(out=od_[:], in0=ep_[:], in1=odT[:, b, :],
                                        op=OP.mult)
                nc.vector.tensor_scalar(out=wT[:, b, :], in0=od_[:],
                                        scalar1=cinv[:], scalar2=None,
                                        op0=OP.mult)
            # row sums r[i] = sum_j wT[j,i] via ones-matmuls, then rinv
            onescol = tl.tile([128, 1], f32)
            nc.vector.memset(onescol[:], 1.0)
            for c in range(TS):
                rp = tps.tile([128, 1], f32, tag="rp")
                for b in range(6):
                    nc.tensor.matmul(out=rp[:], lhsT=wT[:, b, 128 * c:128 * (c + 1)],
                                     rhs=onescol[:], start=(b == 0), stop=(b == 5))
                rcl = tw.tile([128, 1], f32, tag="rcl")
                nc.vector.tensor_scalar(out=rcl[:], in0=rp[:], scalar1=1e-12,
                                        scalar2=None, op0=OP.max)
                nc.vector.reciprocal(out=rinv[:, c:c + 1], in_=rcl[:])

            # aggrT[d, i] : aggr_raw[i,d] = sum_j wT[j,i] feat[j,d], scaled by rinv[i]
            aggrT = tl.tile([128, N], f32)
            for c in range(TS):
                ap_ = tps.tile([128, 128], f32, tag="ap_")
                for b in range(6):
                    nc.tensor.matmul(out=ap_[:], lhsT=wT[:, b, 128 * c:128 * (c + 1)],
                                     rhs=featblk[:, b, :], start=(b == 0),
                                     stop=(b == 5))
                agg_sb = tw.tile([128, 128], f32, tag="agg_sb")
                nc.vector.tensor_scalar(out=agg_sb[:], in0=ap_[:],
                                        scalar1=rinv[:, c:c + 1], scalar2=None,
                                        op0=OP.mult)
                atp = tps.tile([128, 128], f32, tag="atp")
                nc.tensor.transpose(atp[:], agg_sb[:], ident[:])
                nc.scalar.copy(out=aggrT[:, 128 * c:128 * (c + 1)], in_=atp[:])

            # ---------- NodeNet ----------
            nw0T = tl.tile([128, 2, 130], f32)
            nbn0a = tl.tile([128, 2], f32)
            nbn0b = tl.tile([2, 2], f32)
            nw1Ta = tl.tile([128, 65], f32)
            nw1Tb = tl.tile([2, 65], f32)
            nbn1 = tl.tile([65, 2], f32)
            nc.sync.dma_start(out=nw0T[:], in_=nw0T_in[:])
            nc.sync.dma_start(out=nbn0a[:], in_=nbn0_in[0:128, :])
            nc.sync.dma_start(out=nbn0b[:], in_=nbn0_in[128:130, :])
            nc.sync.dma_start(out=nw1Ta[:], in_=nw1T_in[0:128, :])
            nc.sync.dma_start(out=nw1Tb[:], in_=nw1T_in[128:130, :])
            nc.sync.dma_start(out=nbn1[:], in_=nbn1_in[:])

            h2a = tl.tile([128, N], f32)   # first 128 channels of h2
            h2b = tl.tile([2, N], f32)     # last 2 channels
            suma = tl.tile([128, 4], f32)
            sumb = tl.tile([2, 4], f32)
            aggrT_ms = None
            if STAGE < 4:
                for _t in (h2a, h2b, suma, sumb, aggrT):
                    nc.vector.memset(_t[:], 0.5)
            for jc in range(TS2):
                pa = tps.tile([128, F], f32, tag="pa")
                pb = tps.tile([2, F], f32, tag="pb")
                sl = slice(jc * F, (jc + 1) * F)
                for kb, rhs in ((0, featT), (1, aggrT)):
                    nc.tensor.matmul(out=pa[:], lhsT=nw0T[:, kb, 0:128],
                                     rhs=rhs[:, sl], start=(kb == 0),
                                     stop=(kb == 1))
                    nc.tensor.matmul(out=pb[:], lhsT=nw0T[:, kb, 128:130],
                                     rhs=rhs[:, sl], start=(kb == 0),
                                     stop=(kb == 1))
                nc.scalar.activation(out=h2a[:, sl], in_=pa[:], func=AF.Identity,
                                     bias=0.0, scale=1.0,
                                     accum_out=suma[:, jc:jc + 1])
                nc.scalar.activation(out=h2b[:, sl], in_=pb[:], func=AF.Identity,
                                     bias=0.0, scale=1.0,
                                     accum_out=sumb[:, jc:jc + 1])
            scr2a = tw.tile([128, N], f32, tag="scr2a")
            nc.vector.tensor_tensor(out=scr2a[:], in0=h2a[:], in1=h2a[:],
                                    op=OP.mult)
            nc.vector.tensor_reduce(out=suma[:, 2:3], in_=scr2a[:], axis=AX,
                                    op=OP.add)
            scr2b = tw.tile([2, N], f32, tag="scr2b")
            nc.vector.tensor_tensor(out=scr2b[:], in0=h2b[:], in1=h2b[:],
                                    op=OP.mult)
            nc.vector.tensor_reduce(out=sumb[:, 2:3], in_=scr2b[:], axis=AX,
                                    op=OP.add)

            def bn_apply(dst, src, sums, gam, bet, nrows, relu, jc2=2):
                # sums[:,0:2]=partial sums, sums[:,2]=sumsq -> scale/bias
                m_ = tw.tile([nrows, 1], f32, tag=f"m_{nrows}_{relu}")
                nc.vector.tensor_reduce(out=m_[:], in_=sums[:, 0:2], axis=AX,
                                        op=OP.add)
                nc.vector.tensor_scalar(out=m_[:], in0=m_[:], scalar1=1.0 / N,
                                        scalar2=None, op0=OP.mult)
                e2_ = tw.tile([nrows, 1], f32, tag=f"e2_{nrows}_{relu}")
                nc.vector.tensor_scalar(out=e2_[:], in0=sums[:, 2:3],
                                        scalar1=1.0 / N, scalar2=None,
                                        op0=OP.mult)
                mq = tw.tile([nrows, 1], f32, tag=f"mq_{nrows}_{relu}")
                nc.vector.tensor_tensor(out=mq[:], in0=m_[:], in1=m_[:],
                                        op=OP.mult)
                vr = tw.tile([nrows, 1], f32, tag=f"vr_{nrows}_{relu}")
                nc.vector.tensor_tensor(out=vr[:], in0=e2_[:], in1=mq[:],
                                        op=OP.subtract)
                sdt = tw.tile([nrows, 1], f32, tag=f"sd_{nrows}_{relu}")
                nc.scalar.activation(out=sdt[:], in_=vr[:], func=AF.Sqrt,
                                     bias=epscol[0:nrows, :], scale=1.0)
                iv = tw.tile([nrows, 1], f32, tag=f"iv_{nrows}_{relu}")
                nc.vector.reciprocal(out=iv[:], in_=sdt[:])
                ss = tw.tile([nrows, 1], f32, tag=f"ss_{nrows}_{relu}")
                nc.vector.tensor_tensor(out=ss[:], in0=gam, in1=iv[:], op=OP.mult)
                msx = tw.tile([nrows, 1], f32, tag=f"msx_{nrows}_{relu}")
                nc.vector.tensor_tensor(out=msx[:], in0=m_[:], in1=ss[:],
                                        op=OP.mult)
                tt_ = tw.tile([nrows, 1], f32, tag=f"tt_{nrows}_{relu}")
                nc.vector.tensor_tensor(out=tt_[:], in0=bet, in1=msx[:],
                                        op=OP.subtract)
                fn = AF.Lrelu if relu else AF.Identity
                nc.scalar.activation(out=dst, in_=src, func=fn, alpha=NEG,
                                     bias=tt_[:], scale=ss[:])

            a2a = tl.tile([128, N], f32)
            a2b = tl.tile([2, N], f32)
            bn_apply(a2a[:], h2a[:], suma, nbn0a[:, 0:1], nbn0a[:, 1:2],
                     128, True)
            bn_apply(a2b[:], h2b[:], sumb, nbn0b[:, 0:1], nbn0b[:, 1:2],
                     2, True)

            h2c = tl.tile([65, N], f32)
            sumc = tl.tile([65, 4], f32)
            if STAGE < 4:
                nc.vector.memset(h2c[:], 0.5)
                nc.vector.memset(sumc[:], 0.5)
            logT_needs_ms = STAGE < 4
            for jc in range(TS2):
                pc = tps.tile([65, F], f32, tag="pc")
                sl = slice(jc * F, (jc + 1) * F)
                nc.tensor.matmul(out=pc[:], lhsT=nw1Ta[:], rhs=a2a[:, sl],
                                 start=True, stop=False)
                nc.tensor.matmul(out=pc[:], lhsT=nw1Tb[:], rhs=a2b[:, sl],
                                 start=False, stop=True)
                nc.scalar.activation(out=h2c[:, sl], in_=pc[:], func=AF.Identity,
                                     bias=0.0, scale=1.0,
                                     accum_out=sumc[:, jc:jc + 1])
            scr2c = tw.tile([65, N], f32, tag="scr2c")
            nc.vector.tensor_tensor(out=scr2c[:], in0=h2c[:], in1=h2c[:],
                                    op=OP.mult)
            nc.vector.tensor_reduce(out=sumc[:, 2:3], in_=scr2c[:], axis=AX,
                                    op=OP.add)
            logT = tl.tile([65, N], f32)
            bn_apply(logT[:], h2c[:], sumc, nbn1[:, 0:1], nbn1[:, 1:2], 65, False)

            # ---------- outputs: logits = logT^T, sim = simTf^T ----------
            for c in range(TS):
                ltp = tps.tile([128, 65], f32, tag="ltp")
                nc.tensor.transpose(ltp[:], logT[:, 128 * c:128 * (c + 1)],
                                    ident[0:65, 0:65])
                lts = tw.tile([128, 65], f32, tag="lts")
                nc.scalar.copy(out=lts[:], in_=ltp[:])
                nc.sync.dma_start(out=logits_out[128 * c:128 * (c + 1), :],
                                  in_=lts[:])
            for b in range(TS):
                for c in range(TS):
                    smp = tps.tile([128, 128], f32, tag="smp")
                    nc.tensor.transpose(smp[:], simTf[:, b, 128 * c:128 * (c + 1)],
                                        ident[:])
                    sms = tw.tile([128, 128], f32, tag="sms")
                    nc.scalar.copy(out=sms[:], in_=smp[:])
                    nc.sync.dma_start(
                        out=sim_out[128 * c:128 * (c + 1), 128 * b:128 * (b + 1)],
                        in_=sms[:])

    nc.compile()
    return nc


def _host_inputs(init_node_feat, init_node_label, e_w0, e_g0, e_b0, e_w1,
                 e_g1, e_b1, e_wout, e_bout, n_w0, n_g0, n_b0, n_w1, n_g1,
                 n_b1):
    f = np.asarray(init_node_feat, np.float32)
    lab = np.asarray(init_node_label).reshape(-1)
    mask = (lab[:, None] == lab[None, :]).astype(np.float32)
    eye = np.eye(N, dtype=np.float32)
    ep = eye + np.float32(1e-6)
    od = np.float32(1.0) - eye

    def blk6(x):  # [768, W] -> [128, 6, W]
        return np.ascontiguousarray(
            x.reshape(6, 128, -1).transpose(1, 0, 2))

    common = {
        "featT": np.ascontiguousarray(f.T),
        "featblk": blk6(f),
        "w0T": np.ascontiguousarray(
            np.asarray(e_w0, np.float32).reshape(2, 128, 128).transpose(2, 0, 1)),
        "w0": np.ascontiguousarray(
            np.asarray(e_w0, np.float32).reshape(2, 128, 128).transpose(1, 0, 2)),
        "w1T": np.ascontiguousarray(
            np.asarray(e_w1, np.float32).T.reshape(2, 128, 128).transpose(1, 0, 2)),
        "bn0": np.ascontiguousarray(
            np.stack([np.asarray(e_g0, np.float32).reshape(2, 128),
                      np.asarray(e_b0, np.float32).reshape(2, 128)],
                     axis=2).transpose(1, 0, 2)),
        "bn1": np.ascontiguousarray(
            np.stack([np.asarray(e_g1, np.float32),
                      np.asarray(e_b1, np.float32)], axis=1)),
        "woutT": np.ascontiguousarray(np.asarray(e_wout, np.float32).reshape(1, 128).T),
        "bout": np.asarray(e_bout, np.float32).reshape(1, 1),
        "maskT": blk6(mask),
        "epT": blk6(ep),
        "odT": blk6(od),
        "nw0T": np.ascontiguousarray(
            np.asarray(n_w0, np.float32).T.reshape(2, 128, 130).transpose(1, 0, 2)),
        "nbn0": np.ascontiguousarray(
            np.stack([np.asarray(n_g0, np.float32),
                      np.asarray(n_b0, np.float32)], axis=1)),
        "nw1T": np.ascontiguousarray(np.asarray(n_w1, np.float32).T),
        "nbn1": np.ascontiguousarray(
            np.stack([np.asarray(n_g1, np.float32),
                      np.asarray(n_b1, np.float32)], axis=1)),
        "ident": np.eye(128, dtype=np.float32),
    }
    in_maps = []
    for k in range(NCORES):
        rows = f[R * k:R * (k + 1)]
        m = dict(common)
        m["bcastrows"] = np.ascontiguousarray(
            np.repeat(rows, 128, axis=0).reshape(R * 128, 128))
        m["myrowsT"] = np.ascontiguousarray(rows.T)
        in_maps.append(m)
    return in_maps


def kernel(**inputs):
    from concourse.bass_utils import run_bass_kernel_spmd
    if "nc" not in _cached:
        _cached["nc"] = _build()
    nc = _cached["nc"]
    in_maps = _host_inputs(**inputs)
    res = run_bass_kernel_spmd(nc, in_maps, list(range(NCORES)))
    r0 = res.results[0]
    logits = np.asarray(r0["logits"], np.float32)
    sim = np.asarray(r0["sim"], np.float32)
    return logits, sim


# revision 16
# speedup vs baseline: 1.1331x; 1.1331x over previous
"""Trainium2 Bass kernel for nn_ClassifierGNN (gnn_message_passing).

Strategy (8 NeuronCores, rows of the NxN pairwise tensor sharded 96/core):
  Pass A: per-core covariance C = sum_ij x x^T and mean of x (bf16 on PE)
          -> AllReduce -> BN0 stats computed analytically on device.
  Pass B: stream x_ij tiles -> conv0 (f32r) -> fused BN+LeakyReLU (ACT)
          -> conv1 (f32r) -> h1 kept resident in SBUF as bf16 + h1 stats
          -> AllReduce of BN1 stats.
  Pass C: BN1+LeakyReLU on h1, conv_out as column-writes building simT
          on PSUM, sigmoid -> per-core simT block.
  AllGather simT blocks; tail (edge normalize + NodeNet + its BNs)
  computed redundantly per core in transposed layout, fully local.
"""

import numpy as np

N, D, EF, C2, NC_ = 768, 128, 128, 256, 65
NCORES, R = 8, 96
F = 384  # j-chunk for pass B
EPS = 1e-5
NEG = 0.01

_cached = {}


def _build():
    import os
    from concourse import bacc, tile, mybir
    from concourse.dt import dt
    NOCOLL = bool(int(os.environ.get("KNOCOLL", "0")))
    STAGE = int(os.environ.get("KSTAGE", "4"))
    KPA = os.environ.get("KPA", "")

    AF = mybir.ActivationFunctionType
    OP = mybir.AluOpType
    f32, bf16, f32r = dt.float32, dt.bfloat16, dt.float32r
    AX = mybir.AxisListType.X

    nc = bacc.Bacc("TRN2", target_bir_lowering=False, debug=False,
                   num_devices=NCORES)

    def din(name, shape):
        return nc.dram_tensor(name, list(shape), mybir.dt.float32,
                              kind="ExternalInput")

    featT_in = din("featT", [128, N])
    featblk_in = din("featblk", [128, 6, 128])
    bcast_in = din("bcastrows", [R * 128, 128])
    myrowsT_in = din("myrowsT", [128, R])
    w0T_in = din("w0T", [128, 2, 128])
    w0_in = din("w0", [128, 2, 128])
    w1T_in = din("w1T", [128, 2, 128])
    bn0_in = din("bn0", [128, 2, 2])
    bn1_in = din("bn1", [128, 2])
    woutT_in = din("woutT", [128, 1])
    bout_in = din("bout", [1, 1])
    maskT_in = din("maskT", [128, 6, N])
    epT_in = din("epT", [128, 6, N])
    odT_in = din("odT", [128, 6, N])
    nw0T_in = din("nw0T", [128, 2, 130])
    nbn0_in = din("nbn0", [130, 2])
    nw1T_in = din("nw1T", [130, 65])
    nbn1_in = din("nbn1", [65, 2])
    ident_in = din("ident", [128, 128])

    logits_out = nc.dram_tensor("logits", [N, NC_], mybir.dt.float32,
                                kind="ExternalOutput")
    sim_out = nc.dram_tensor("sim", [N, N], mybir.dt.float32,
                             kind="ExternalOutput")

    # collective bounce buffers (internal DRAM)
    ar0_src = nc.dram_tensor("ar0_src", [128, 130], mybir.dt.float32)
    ar0_dst = nc.dram_tensor("ar0_dst", [128, 130], mybir.dt.float32)
    ar1_src = nc.dram_tensor("ar1_src", [128, 2], mybir.dt.float32)
    ar1_dst = nc.dram_tensor("ar1_dst", [128, 2], mybir.dt.float32)
    ag_src = nc.dram_tensor("ag_src", [N, R], mybir.dt.float32)
    ag_dst = nc.dram_tensor("ag_dst", [NCORES * N, R], mybir.dt.float32)

    NSQ = float(N) * float(N)

    from contextlib import ExitStack
    with tile.TileContext(nc) as tc:
      with tc.tile_pool(name="res", bufs=1) as res, \
           tc.tile_pool(name="stat", bufs=1) as stat:
        with tc.tile_pool(name="h1res", bufs=1) as h1pool, \
             tc.tile_pool(name="work", bufs=2) as work:
            pstack = ExitStack()
            psA = pstack.enter_context(
                tc.tile_pool(name="psA", bufs=1, space="PSUM"))
            ps = psA

            # ---------- resident loads ----------
            featT = res.tile([128, N], f32)
            featblk = res.tile([128, 6, 128], f32)
            myrowsT = res.tile([128, R], f32)
            w0T = res.tile([128, 2, 128], f32)
            w0r_ = res.tile([128, 2, 128], f32r)
            w0rows = res.tile([128, 2, 128], f32)
            w1T = res.tile([128, 2, 128], f32)
            w1r_ = res.tile([128, 2, 128], f32r)
            bn0 = res.tile([128, 2, 2], f32)
            bn1 = res.tile([128, 2], f32)
            woutT = res.tile([128, 1], f32)
            woutb = res.tile([128, 1], bf16)
            ident = res.tile([128, 128], f32)
            for t, src in [(featT, featT_in), (featblk, featblk_in),
                           (myrowsT, myrowsT_in),
                           (w0T, w0T_in), (w0rows, w0_in), (w1T, w1T_in),
                           (bn0, bn0_in), (bn1, bn1_in), (woutT, woutT_in),
                           (ident, ident_in)]:
                nc.sync.dma_start(out=t[:], in_=src[:])
            negrT = res.tile([128, R], f32)
            nc.vector.tensor_scalar(out=negrT[:], in0=myrowsT[:], scalar1=-1.0,
                                    scalar2=None, op0=OP.mult)
            for b in range(2):
                nc.vector.tensor_copy(out=w0r_[:, b, :], in_=w0T[:, b, :])
                nc.vector.tensor_copy(out=w1r_[:, b, :], in_=w1T[:, b, :])
            nc.vector.tensor_copy(out=woutb[:], in_=woutT[:])
            ones1 = res.tile([1, 128], f32)
            nc.vector.memset(ones1[:], 1.0)
            onesb = res.tile([128, 1], bf16)
            nc.vector.memset(onesb[:], 1.0)
            epscol = res.tile([128, 1], f32)
            nc.vector.memset(epscol[:], EPS)

            # ---------- Pass A: C = sum x x^T, mx = sum x  (bf16) ----------
            Cp = psA.tile([128, 128], f32)
            mxp = psA.tile([128, 1], f32)
            nmm = 0
            NACC = R * 6 if "shortacc" not in KPA else 12
            for i in range(R if STAGE >= 1 else 0):
                bcs = work.tile([128, 128], f32, tag="bcs")
                if "nodma" in KPA:
                    nc.vector.memset(bcs[:], 0.5)
                else:
                    nc.sync.dma_start(out=bcs[:], in_=bcast_in[128 * i:128 * (i + 1), :])
                for b in range(6):
                    xb = work.tile([128, 128], bf16, tag="xb")
                    if "nogen" in KPA:
                        nc.vector.memset(xb[:], 0.25)
                    else:
                        xs = work.tile([128, 128], f32, tag="xs")
                        nc.vector.tensor_tensor(out=xs[:], in0=featblk[:, b, :],
                                                in1=bcs[:], op=OP.subtract)
                        nc.scalar.activation(out=xb[:], in_=xs[:], func=AF.Abs,
                                             bias=0.0, scale=1.0)
                    acc_i = nmm % NACC
                    first = acc_i == 0
                    last = (acc_i == NACC - 1) or (i == R - 1 and b == 5)
                    if "nomm" not in KPA:
                        nc.tensor.matmul(out=Cp[:], lhsT=xb[:], rhs=xb[:],
                                         start=first, stop=last)
                    if "nomx" not in KPA and "nomm" not in KPA:
                        nc.tensor.matmul(out=mxp[:], lhsT=xb[:], rhs=onesb[:],
                                         start=first, stop=last)
                    nmm += 1
            if "nomm" in KPA:
                nc.tensor.matmul(out=Cp[:], lhsT=onesb[:].bitcast(bf16), rhs=onesb[:],
                                 start=True, stop=True) if False else None
                nc.vector.memset(Cp[:], 0.5)
                nc.vector.memset(mxp[:], 0.5)
            elif "nomx" in KPA:
                nc.vector.memset(mxp[:], 0.5)
            ar0sb = stat.tile([128, 130], f32)
            nc.vector.tensor_copy(out=ar0sb[:, 0:128], in_=Cp[:])
            nc.vector.tensor_copy(out=ar0sb[:, 128:129], in_=mxp[:])
            nc.vector.memset(ar0sb[:, 129:130], 0.0)
            nc.sync.dma_start(out=ar0_src[:], in_=ar0sb[:])
            if NOCOLL:
                nc.sync.dma_start(out=ar0_dst[:], in_=ar0_src[:])
            else:
                nc.gpsimd.collective_compute(
                    "AllReduce", OP.add, replica_groups=[list(range(NCORES))],
                    ins=[ar0_src.ap().opt()], outs=[ar0_dst.ap().opt()])
            CR = stat.tile([128, 130], f32)
            nc.sync.dma_start(out=CR[:], in_=ar0_dst[:])
            pstack.close()
            pstack = ExitStack()
            ps = pstack.enter_context(
                tc.tile_pool(name="ps0", bufs=1, space="PSUM"))

            # ---------- BN0 stats ----------
            s0 = stat.tile([128, 2], f32)
            t0 = stat.tile([128, 2], f32)
            for b in range(2):
                m0p = ps.tile([128, 1], f32, tag="m0p")
                nc.tensor.matmul(out=m0p[:], lhsT=w0T[:, b, :],
                                 rhs=CR[:, 128:129], start=True, stop=True)
                M1p = ps.tile([128, 128], f32, tag="M1p")
                nc.tensor.matmul(out=M1p[:], lhsT=w0T[:, b, :],
                                 rhs=CR[:, 0:128], start=True, stop=True)
                e2 = stat.tile([128, 1], f32, tag=f"e2_{b}")
                scr = work.tile([128, 128], f32, tag="scr")
                nc.vector.tensor_tensor(out=scr[:], in0=M1p[:],
                                        in1=w0rows[:, b, :], op=OP.mult)
                nc.vector.tensor_reduce(out=e2[:], in_=scr[:], axis=AX,
                                        op=OP.add)
                nc.vector.tensor_scalar(out=e2[:], in0=e2[:],
                                        scalar1=1.0 / NSQ, scalar2=None,
                                        op0=OP.mult)
                m0 = stat.tile([128, 1], f32, tag=f"m0_{b}")
                nc.vector.tensor_scalar(out=m0[:], in0=m0p[:],
                                        scalar1=1.0 / NSQ, scalar2=None,
                                        op0=OP.mult)
                msq = stat.tile([128, 1], f32, tag=f"msq_{b}")
                nc.vector.tensor_tensor(out=msq[:], in0=m0[:], in1=m0[:],
                                        op=OP.mult)
                var = stat.tile([128, 1], f32, tag=f"var_{b}")
                nc.vector.tensor_tensor(out=var[:], in0=e2[:], in1=msq[:],
                                        op=OP.subtract)
                sd = stat.tile([128, 1], f32, tag=f"sd_{b}")
                nc.scalar.activation(out=sd[:], in_=var[:], func=AF.Sqrt,
                                     bias=epscol[:], scale=1.0)
                inv = stat.tile([128, 1], f32, tag=f"inv_{b}")
                nc.vector.reciprocal(out=inv[:], in_=sd[:])
                # s = gamma * inv ; t = beta - m * s
                nc.vector.tensor_tensor(out=s0[:, b:b + 1], in0=bn0[:, b, 0:1],
                                        in1=inv[:], op=OP.mult)
                ms = stat.tile([128, 1], f32, tag=f"ms_{b}")
                nc.vector.tensor_tensor(out=ms[:], in0=m0[:],
                                        in1=s0[:, b:b + 1], op=OP.mult)
                nc.vector.tensor_tensor(out=t0[:, b:b + 1], in0=bn0[:, b, 1:2],
                                        in1=ms[:], op=OP.subtract)

            # ---------- Pass B ----------
            pstack.close()
            pstack = ExitStack()
            ps = pstack.enter_context(
                tc.tile_pool(name="psB", bufs=2, space="PSUM"))
            h1res = h1pool.tile([128, R, N], bf16)
            a0acc = stat.tile([128, 2, 2 * R], f32)
            s2acc = stat.tile([128, 2 * R], f32)
            if STAGE < 2:
                nc.vector.memset(a0acc[:], 1.0)
                nc.vector.memset(s2acc[:], 2.0)
            for i in range(R if STAGE >= 2 else 0):
                for jc in range(2):
                    it = i * 2 + jc
                    xsub = work.tile([128, F], f32, tag="xsub")
                    nc.gpsimd.tensor_tensor(
                        out=xsub[:], in0=featT[:, jc * F:(jc + 1) * F],
                        in1=myrowsT[:, i:i + 1].to_broadcast((128, F)),
                        op=OP.subtract)
                    xr = work.tile([128, F], f32r, tag="xr")
                    nc.vector.scalar_tensor_tensor(
                        out=xr[:], in0=xsub[:], scalar=-1.0, in1=xsub[:],
                        op0=OP.mult, op1=OP.max)
                    h0p = [ps.tile([128, F], f32, tag="h0pa", name="h0pa"),
                           ps.tile([128, F], f32, tag="h0pb", name="h0pb")]
                    for b in range(2):
                        nc.tensor.matmul(out=h0p[b][:], lhsT=w0r_[:, b, :],
                                         rhs=xr[:], start=True, stop=True)
                    a0 = work.tile([128, 2, F], f32r, tag="a0")
                    for b in range(2):
                        nc.scalar.activation(out=a0[:, b, :], in_=h0p[b][:],
                                             func=AF.Lrelu, alpha=NEG,
                                             bias=t0[:, b:b + 1],
                                             scale=s0[:, b:b + 1],
                                             accum_out=a0acc[:, b, it:it + 1])
                    h1p = ps.tile([128, F], f32, tag="h1p")
                    nc.tensor.matmul(out=h1p[:], lhsT=w1r_[:, 0, :],
                                     rhs=a0[:, 0, :], start=True, stop=False)
                    nc.tensor.matmul(out=h1p[:], lhsT=w1r_[:, 1, :],
                                     rhs=a0[:, 1, :], start=False, stop=True)
                    nc.vector.tensor_copy(out=h1res[:, i, jc * F:(jc + 1) * F],
                                          in_=h1p[:])
                    sqs = work.tile([128, F], bf16, tag="sqs")
                    nc.vector.tensor_tensor(
                        out=sqs[:], in0=h1res[:, i, jc * F:(jc + 1) * F],
                        in1=h1res[:, i, jc * F:(jc + 1) * F], op=OP.mult)
                    nc.vector.tensor_reduce(out=s2acc[:, it:it + 1], in_=sqs[:],
                                            axis=AX, op=OP.add)

            # Sum(h1) = w1^T . Sum(a0) per block (tiny)
            a0col = stat.tile([128, 2], f32)
            for b in range(2):
                nc.vector.tensor_reduce(out=a0col[:, b:b + 1],
                                        in_=a0acc[:, b, :], axis=AX, op=OP.add)
            s1p = ps.tile([128, 1], f32, tag="s1p")
            nc.tensor.matmul(out=s1p[:], lhsT=w1T[:, 0, :], rhs=a0col[:, 0:1],
                             start=True, stop=False)
            nc.tensor.matmul(out=s1p[:], lhsT=w1T[:, 1, :], rhs=a0col[:, 1:2],
                             start=False, stop=True)
            ar1sb = stat.tile([128, 2], f32)
            nc.vector.tensor_copy(out=ar1sb[:, 0:1], in_=s1p[:])
            nc.vector.tensor_reduce(out=ar1sb[:, 1:2], in_=s2acc[:], axis=AX,
                                    op=OP.add)
            nc.sync.dma_start(out=ar1_src[:], in_=ar1sb[:])
            if NOCOLL:
                nc.sync.dma_start(out=ar1_dst[:], in_=ar1_src[:])
            else:
                nc.gpsimd.collective_compute(
                    "AllReduce", OP.add, replica_groups=[list(range(NCORES))],
                    ins=[ar1_src.ap().opt()], outs=[ar1_dst.ap().opt()])
            SR = stat.tile([128, 2], f32)
            nc.sync.dma_start(out=SR[:], in_=ar1_dst[:])

            # BN1 stats
            s1 = stat.tile([128, 1], f32)
            t1 = stat.tile([128, 1], f32)
            m1 = stat.tile([128, 1], f32)
            nc.vector.tensor_scalar(out=m1[:], in0=SR[:, 0:1], scalar1=1.0 / NSQ,
                                    scalar2=None, op0=OP.mult)
            msq1 = stat.tile([128, 1], f32)
            nc.vector.tensor_tensor(out=msq1[:], in0=m1[:], in1=m1[:], op=OP.mult)
            var1 = stat.tile([128, 1], f32)
            nc.vector.tensor_scalar(out=var1[:], in0=SR[:, 1:2], scalar1=1.0 / NSQ,
                                    scalar2=None, op0=OP.mult)
            nc.vector.tensor_tensor(out=var1[:], in0=var1[:], in1=msq1[:],
                                    op=OP.subtract)
            sd1 = stat.tile([128, 1], f32)
            nc.scalar.activation(out=sd1[:], in_=var1[:], func=AF.Sqrt,
                                 bias=epscol[:], scale=1.0)
            inv1 = stat.tile([128, 1], f32)
            nc.vector.reciprocal(out=inv1[:], in_=sd1[:])
            nc.vector.tensor_tensor(out=s1[:], in0=bn1[:, 0:1], in1=inv1[:],
                                    op=OP.mult)
            ms1 = stat.tile([128, 1], f32)
            nc.vector.tensor_tensor(out=ms1[:], in0=m1[:], in1=s1[:], op=OP.mult)
            nc.vector.tensor_tensor(out=t1[:], in0=bn1[:, 1:2], in1=ms1[:],
                                    op=OP.subtract)

            pstack.close()
            pstack = ExitStack()
            ps = pstack.enter_context(
                tc.tile_pool(name="psC", bufs=2, space="PSUM"))

            # bout broadcast to [128,1]
            bout_sb1 = stat.tile([1, 1], f32)
            nc.sync.dma_start(out=bout_sb1[:], in_=bout_in[:])
            boutp = ps.tile([128, 1], f32, tag="boutp")
            nc.tensor.matmul(out=boutp[:], lhsT=ones1[:], rhs=bout_sb1[:],
                             start=True, stop=True)
            boutc = stat.tile([128, 1], f32)
            nc.vector.tensor_copy(out=boutc[:], in_=boutp[:])

            # ---------- Pass C: simT = sigmoid(wout . lrelu(bn1(h1))) ----------
            simT = stat.tile([128, 6, R], f32)
            if STAGE < 3:
                nc.vector.memset(simT[:], 0.5)
            stps = []
            for bj in range(6 if STAGE >= 3 else 0):
                stp = ps.tile([128, R], f32, tag=f"stp{bj}", name=f"stp{bj}",
                              bufs=1)
                stps.append(stp)
                for i in range(R):
                    a1 = work.tile([128, 128], bf16, tag="a1")
                    nc.scalar.activation(out=a1[:],
                                         in_=h1res[:, i, bj * 128:(bj + 1) * 128],
                                         func=AF.Lrelu, alpha=NEG, bias=t1[:],
                                         scale=s1[:])
                    nc.tensor.matmul(out=stp[:, i:i + 1], lhsT=a1[:],
                                     rhs=woutb[:], start=True, stop=True)
            for bj in range(6 if STAGE >= 3 else 0):
                nc.scalar.activation(out=simT[:, bj, :], in_=stps[bj][:],
                                     func=AF.Sigmoid, bias=boutc[:], scale=1.0)

            # ---------- AllGather simT ----------
            for b in range(6 if STAGE >= 3 else 0):
                nc.sync.dma_start(out=ag_src[128 * b:128 * (b + 1), :],
                                  in_=simT[:, b, :])
            if NOCOLL:
                nc.sync.dma_start(out=ag_dst[0:N, :], in_=ag_src[:])
            else:
                nc.gpsimd.collective_compute(
                    "AllGather", OP.bypass, replica_groups=[list(range(NCORES))],
                    ins=[ag_src.ap().opt()], outs=[ag_dst.ap().opt()])
            pstack.close()

        # ---------- tail (h1res freed) ----------
        with tc.tile_pool(name="tail", bufs=1) as tl, \
             tc.tile_pool(name="twork", bufs=3) as tw, \
             tc.tile_pool(name="tps", bufs=1, space="PSUM") as tps:
            TS = 6 if STAGE >= 4 else 0
            TS2 = 2 if STAGE >= 4 else 0
            simTf = tl.tile([128, 6, N], f32)
            for b in range(TS):
                for k in range(NCORES):
                    nc.sync.dma_start(
                        out=simTf[:, b, R * k:R * (k + 1)],
                        in_=ag_dst[N * k + 128 * b:N * k + 128 * (b + 1), :])
            maskT = tl.tile([128, 6, N], f32)
            epT = tl.tile([128, 6, N], f32)
            odT = tl.tile([128, 6, N], f32)
            nc.sync.dma_start(out=maskT[:], in_=maskT_in[:])
            nc.sync.dma_start(out=epT[:], in_=epT_in[:])
            nc.sync.dma_start(out=odT[:], in_=odT_in[:])

            # edge_pre = mask*sim + ep ; colsum over free(i) ; w = od*edge_pre/colsum[j]
            wT = tl.tile([128, 6, N], f32)
            rinv = tl.tile([128, 6], f32)
            colp = tl.tile([128, 6], f32)
            if STAGE < 4:
                for _t in (simTf, wT, rinv, colp):
                    nc.vector.memset(_t[:], 0.25)
            for b in range(TS):
                ep_ = tw.tile([128, N], f32, tag="ep_")
                nc.vector.tensor_tensor(out=ep_[:], in0=maskT[:, b, :],
                                        in1=simTf[:, b, :], op=OP.mult)
                nc.vector.tensor_tensor(out=ep_[:], in0=ep_[:], in1=epT[:, b, :],
                                        op=OP.add)
                nc.vector.tensor_reduce(out=colp[:, b:b + 1], in_=ep_[:],
                                        axis=AX, op=OP.add)
                cinv = tw.tile([128, 1], f32, tag="cinv")
                nc.vector.reciprocal(out=cinv[:], in_=colp[:, b:b + 1])
                od_ = tw.tile([128, N], f32, tag="od_")
                nc.vector.tensor_tensor(out=od_[:], in0=ep_[:], in1=odT[:, b, :],
                                        op=OP.mult)
                nc.vector.tensor_scalar(out=wT[:, b, :], in0=od_[:],
                                        scalar1=cinv[:], scalar2=None,
                                        op0=OP.mult)
            # row sums r[i] = sum_j wT[j,i] via ones-matmuls, then rinv
            onescol = tl.tile([128, 1], f32)
            nc.vector.memset(onescol[:], 1.0)
            for c in range(TS):
                rp = tps.tile([128, 1], f32, tag="rp")
                for b in range(6):
                    nc.tensor.matmul(out=rp[:], lhsT=wT[:, b, 128 * c:128 * (c + 1)],
                                     rhs=onescol[:], start=(b == 0), stop=(b == 5))
                rcl = tw.tile([128, 1], f32, tag="rcl")
                nc.vector.tensor_scalar(out=rcl[:], in0=rp[:], scalar1=1e-12,
                                        scalar2=None, op0=OP.max)
                nc.vector.reciprocal(out=rinv[:, c:c + 1], in_=rcl[:])

            # aggrT[d, i] : aggr_raw[i,d] = sum_j wT[j,i] feat[j,d], scaled by rinv[i]
            aggrT = tl.tile([128, N], f32)
            for c in range(TS):
                ap_ = tps.tile([128, 128], f32, tag="ap_")
                for b in range(6):
                    nc.tensor.matmul(out=ap_[:], lhsT=wT[:, b, 128 * c:128 * (c + 1)],
                                     rhs=featblk[:, b, :], start=(b == 0),
                                     stop=(b == 5))
                agg_sb = tw.tile([128, 128], f32, tag="agg_sb")
                nc.vector.tensor_scalar(out=agg_sb[:], in0=ap_[:],
                                        scalar1=rinv[:, c:c + 1], scalar2=None,
                                        op0=OP.mult)
                atp = tps.tile([128, 128], f32, tag="atp")
                nc.tensor.transpose(atp[:], agg_sb[:], ident[:])
                nc.vector.tensor_copy(out=aggrT[:, 128 * c:128 * (c + 1)], in_=atp[:])

            # ---------- NodeNet ----------
            nw0T = tl.tile([128, 2, 130], f32)
            nbn0a = tl.tile([128, 2], f32)
            nbn0b = tl.tile([2, 2], f32)
            nw1Ta = tl.tile([128, 65], f32)
            nw1Tb = tl.tile([2, 65], f32)
            nbn1 = tl.tile([65, 2], f32)
            nc.sync.dma_start(out=nw0T[:], in_=nw0T_in[:])
            nc.sync.dma_start(out=nbn0a[:], in_=nbn0_in[0:128, :])
            nc.sync.dma_start(out=nbn0b[:], in_=nbn0_in[128:130, :])
            nc.sync.dma_start(out=nw1Ta[:], in_=nw1T_in[0:128, :])
            nc.sync.dma_start(out=nw1Tb[:], in_=nw1T_in[128:130, :])
            nc.sync.dma_start(out=nbn1[:], in_=nbn1_in[:])

            h2a = tl.tile([128, N], f32)   # first 128 channels of h2
            h2b = tl.tile([2, N], f32)     # last 2 channels
            suma = tl.tile([128, 4], f32)
            sumb = tl.tile([2, 4], f32)
            aggrT_ms = None
            if STAGE < 4:
                for _t in (h2a, h2b, suma, sumb, aggrT):
                    nc.vector.memset(_t[:], 0.5)
            for jc in range(TS2):
                pa = tps.tile([128, F], f32, tag="pa")
                pb = tps.tile([2, F], f32, tag="pb")
                sl = slice(jc * F, (jc + 1) * F)
                for kb, rhs in ((0, featT), (1, aggrT)):
                    nc.tensor.matmul(out=pa[:], lhsT=nw0T[:, kb, 0:128],
                                     rhs=rhs[:, sl], start=(kb == 0),
                                     stop=(kb == 1))
                    nc.tensor.matmul(out=pb[:], lhsT=nw0T[:, kb, 128:130],
                                     rhs=rhs[:, sl], start=(kb == 0),
                                     stop=(kb == 1))
                nc.scalar.activation(out=h2a[:, sl], in_=pa[:], func=AF.Identity,
                                     bias=0.0, scale=1.0,
                                     accum_out=suma[:, jc:jc + 1])
                nc.scalar.activation(out=h2b[:, sl], in_=pb[:], func=AF.Identity,
                                     bias=0.0, scale=1.0,
                                     accum_out=sumb[:, jc:jc + 1])
            scr2a = tw.tile([128, N], f32, tag="scr2a")
            nc.vector.tensor_tensor(out=scr2a[:], in0=h2a[:], in1=h2a[:],
                                    op=OP.mult)
            nc.vector.tensor_reduce(out=suma[:, 2:3], in_=scr2a[:], axis=AX,
                                    op=OP.add)
            scr2b = tw.tile([2, N], f32, tag="scr2b")
            nc.vector.tensor_tensor(out=scr2b[:], in0=h2b[:], in1=h2b[:],
                                    op=OP.mult)
            nc.vector.tensor_reduce(out=sumb[:, 2:3], in_=scr2b[:], axis=AX,
                                    op=OP.add)

            def bn_apply(dst, src, sums, gam, bet, nrows, relu, jc2=2):
                # sums[:,0:2]=partial sums, sums[:,2]=sumsq -> scale/bias
                m_ = tw.tile([nrows, 1], f32, tag=f"m_{nrows}_{relu}")
                nc.vector.tensor_reduce(out=m_[:], in_=sums[:, 0:2], axis=AX,
                                        op=OP.add)
                nc.vector.tensor_scalar(out=m_[:], in0=m_[:], scalar1=1.0 / N,
                                        scalar2=None, op0=OP.mult)
                e2_ = tw.tile([nrows, 1], f32, tag=f"e2_{nrows}_{relu}")
                nc.vector.tensor_scalar(out=e2_[:], in0=sums[:, 2:3],
                                        scalar1=1.0 / N, scalar2=None,
                                        op0=OP.mult)
                mq = tw.tile([nrows, 1], f32, tag=f"mq_{nrows}_{relu}")
                nc.vector.tensor_tensor(out=mq[:], in0=m_[:], in1=m_[:],
                                        op=OP.mult)
                vr = tw.tile([nrows, 1], f32, tag=f"vr_{nrows}_{relu}")
                nc.vector.tensor_tensor(out=vr[:], in0=e2_[:], in1=mq[:],
                                        op=OP.subtract)
                sdt = tw.tile([nrows, 1], f32, tag=f"sd_{nrows}_{relu}")
                nc.scalar.activation(out=sdt[:], in_=vr[:], func=AF.Sqrt,
                                     bias=epscol[0:nrows, :], scale=1.0)
                iv = tw.tile([nrows, 1], f32, tag=f"iv_{nrows}_{relu}")
                nc.vector.reciprocal(out=iv[:], in_=sdt[:])
                ss = tw.tile([nrows, 1], f32, tag=f"ss_{nrows}_{relu}")
                nc.vector.tensor_tensor(out=ss[:], in0=gam, in1=iv[:], op=OP.mult)
                msx = tw.tile([nrows, 1], f32, tag=f"msx_{nrows}_{relu}")
                nc.vector.tensor_tensor(out=msx[:], in0=m_[:], in1=ss[:],
                                        op=OP.mult)
                tt_ = tw.tile([nrows, 1], f32, tag=f"tt_{nrows}_{relu}")
                nc.vector.tensor_tensor(out=tt_[:], in0=bet, in1=msx[:],
                                        op=OP.subtract)
                fn = AF.Lrelu if relu else AF.Identity
                nc.scalar.activation(out=dst, in_=src, func=fn, alpha=NEG,
                                     bias=tt_[:], scale=ss[:])

            a2a = tl.tile([128, N], f32)
            a2b = tl.tile([2, N], f32)
            bn_apply(a2a[:], h2a[:], suma, nbn0a[:, 0:1], nbn0a[:, 1:2],
                     128, True)
            bn_apply(a2b[:], h2b[:], sumb, nbn0b[:, 0:1], nbn0b[:, 1:2],
                     2, True)

            h2c = tl.tile([65, N], f32)
            sumc = tl.tile([65, 4], f32)
            if STAGE < 4:
                nc.vector.memset(h2c[:], 0.5)
                nc.vector.memset(sumc[:], 0.5)
            logT_needs_ms = STAGE < 4
            for jc in range(TS2):
                pc = tps.tile([65, F], f32, tag="pc")
                sl = slice(jc * F, (jc + 1) * F)
                nc.tensor.matmul(out=pc[:], lhsT=nw1Ta[:], rhs=a2a[:, sl],
                                 start=True, stop=False)
                nc.tensor.matmul(out=pc[:], lhsT=nw1Tb[:], rhs=a2b[:, sl],
                                 start=False, stop=True)
                nc.scalar.activation(out=h2c[:, sl], in_=pc[:], func=AF.Identity,
                                     bias=0.0, scale=1.0,
                                     accum_out=sumc[:, jc:jc + 1])
            scr2c = tw.tile([65, N], f32, tag="scr2c")
            nc.vector.tensor_tensor(out=scr2c[:], in0=h2c[:], in1=h2c[:],
                                    op=OP.mult)
            nc.vector.tensor_reduce(out=sumc[:, 2:3], in_=scr2c[:], axis=AX,
                                    op=OP.add)
            logT = tl.tile([65, N], f32)
            bn_apply(logT[:], h2c[:], sumc, nbn1[:, 0:1], nbn1[:, 1:2], 65, False)

            # ---------- outputs: logits = logT^T, sim = simTf^T ----------
            for c in range(TS):
                ltp = tps.tile([128, 65], f32, tag="ltp")
                nc.tensor.transpose(ltp[:], logT[:, 128 * c:128 * (c + 1)],
                                    ident[0:65, 0:65])
                lts = tw.tile([128, 65], f32, tag="lts")
                nc.vector.tensor_copy(out=lts[:], in_=ltp[:])
                nc.sync.dma_start(out=logits_out[128 * c:128 * (c + 1), :],
                                  in_=lts[:])
            for b in range(TS):
                for c in range(TS):
                    smp = tps.tile([128, 128], f32, tag="smp")
                    nc.tensor.transpose(smp[:], simTf[:, b, 128 * c:128 * (c + 1)],
                                        ident[:])
                    sms = tw.tile([128, 128], f32, tag="sms")
                    nc.vector.tensor_copy(out=sms[:], in_=smp[:])
                    nc.sync.dma_start(
                        out=sim_out[128 * c:128 * (c + 1), 128 * b:128 * (b + 1)],
                        in_=sms[:])

    nc.compile()
    return nc


def _host_inputs(init_node_feat, init_node_label, e_w0, e_g0, e_b0, e_w1,
                 e_g1, e_b1, e_wout, e_bout, n_w0, n_g0, n_b0, n_w1, n_g1,
                 n_b1):
    f = np.asarray(init_node_feat, np.float32)
    lab = np.asarray(init_node_label).reshape(-1)
    mask = (lab[:, None] == lab[None, :]).astype(np.float32)
    eye = np.eye(N, dtype=np.float32)
    ep = eye + np.float32(1e-6)
    od = np.float32(1.0) - eye

    def blk6(x):  # [768, W] -> [128, 6, W]
        return np.ascontiguousarray(
            x.reshape(6, 128, -1).transpose(1, 0, 2))

    common = {
        "featT": np.ascontiguousarray(f.T),
        "featblk": blk6(f),
        "w0T": np.ascontiguousarray(
            np.asarray(e_w0, np.float32).reshape(2, 128, 128).transpose(2, 0, 1)),
        "w0": np.ascontiguousarray(
            np.asarray(e_w0, np.float32).reshape(2, 128, 128).transpose(1, 0, 2)),
        "w1T": np.ascontiguousarray(
            np.asarray(e_w1, np.float32).T.reshape(2, 128, 128).transpose(1, 0, 2)),
        "bn0": np.ascontiguousarray(
            np.stack([np.asarray(e_g0, np.float32).reshape(2, 128),
                      np.asarray(e_b0, np.float32).reshape(2, 128)],
                     axis=2).transpose(1, 0, 2)),
        "bn1": np.ascontiguousarray(
            np.stack([np.asarray(e_g1, np.float32),
                      np.asarray(e_b1, np.float32)], axis=1)),
        "woutT": np.ascontiguousarray(np.asarray(e_wout, np.float32).reshape(1, 128).T),
        "bout": np.asarray(e_bout, np.float32).reshape(1, 1),
        "maskT": blk6(mask),
        "epT": blk6(ep),
        "odT": blk6(od),
        "nw0T": np.ascontiguousarray(
            np.asarray(n_w0, np.float32).T.reshape(2, 128, 130).transpose(1, 0, 2)),
        "nbn0": np.ascontiguousarray(
            np.stack([np.asarray(n_g0, np.float32),
                      np.asarray(n_b0, np.float32)], axis=1)),
        "nw1T": np.ascontiguousarray(np.asarray(n_w1, np.float32).T),
        "nbn1": np.ascontiguousarray(
            np.stack([np.asarray(n_g1, np.float32),
                      np.asarray(n_b1, np.float32)], axis=1)),
        "ident": np.eye(128, dtype=np.float32),
    }
    in_maps = []
    for k in range(NCORES):
        rows = f[R * k:R * (k + 1)]
        m = dict(common)
        m["bcastrows"] = np.ascontiguousarray(
            np.repeat(rows, 128, axis=0).reshape(R * 128, 128))
        m["myrowsT"] = np.ascontiguousarray(rows.T)
        in_maps.append(m)
    return in_maps


def kernel(**inputs):
    from concourse.bass_utils import run_bass_kernel_spmd
    if "nc" not in _cached:
        _cached["nc"] = _build()
    nc = _cached["nc"]
    in_maps = _host_inputs(**inputs)
    res = run_bass_kernel_spmd(nc, in_maps, list(range(NCORES)))
    r0 = res.results[0]
    logits = np.asarray(r0["logits"], np.float32)
    sim = np.asarray(r0["sim"], np.float32)
    return logits, sim
